# revision 22
# baseline (speedup 1.0000x reference)
"""GAT (3-layer, 4-head) forward on 8 Trainium2 NeuronCores.

Strategy: partition nodes by destination across 8 cores (graph-parallel),
renumber nodes so each core's shard is degree-sorted, route edges to the
dst-owning core in a degree-slot layout (slot = (dst_local partition, round)).
Segment softmax + scatter then reduce to plain PSUM accumulation of
exp-scaled gathered rows via an identity matmul. Attention coefficients
al_src/al_dst are folded into the dense-phase weight matrix as extra output
columns, so one bulk dma_gather per block fetches everything per edge.
Halo exchange of h rows via AllGather each layer.
"""
import sys
sys.path.insert(0, "/opt/trn_rl_repo")
import numpy as np

# ---- problem constants (hardcoded per contest contract) ----
N = 50000
E = 800000
D = 128
H = 4
C = 64
HC = 256
B = 64
OUT = 10
SLOPE = 0.2

NCOR = 8
NLOC = N // NCOR          # 6250
PBLK = 128
NBLK = (NLOC + PBLK - 1) // PBLK      # 49
NLOCP = NBLK * PBLK       # 6272 padded local nodes
NTAB = NCOR * NLOCP       # 50176 global padded table rows
ROW = 320                 # legacy f32 row (unused in bf16 mode)
WCOL = 264                # Waug output cols: 256 h | 4 als | 4 ald
RB = 384                  # bf16 cols per h row (768B): 256 h | 16 al-f32-bits | pad
SPLIT = 32768             # int16 gather index limit (window size)
GCH = 8                   # dma_gather caps at 1024 indices = 8 rounds
F32 = np.float32

# AllGather row chunks (in blocks). The shared gather table is laid out
# chunk-major ([all cores chunk0 | all cores chunk1 | ...]) so each chunked
# AllGather writes one contiguous slice.
AGS = 4
AGB = [0] + [round((s + 1) * NBLK / AGS) for s in range(AGS)]
AGSZ = [(AGB[s + 1] - AGB[s]) * PBLK for s in range(AGS)]   # rows/core/chunk
AGCUM = np.r_[0, np.cumsum(AGSZ)]                           # per-core offsets
AGBASE = [int(NCOR * AGCUM[s]) for s in range(AGS + 1)]     # table offsets


def table_row(c, r):
    """Global gather-table row for local row r of core c (chunk-major)."""
    s = np.searchsorted(AGCUM, r, side="right") - 1
    return AGBASE[s] + c * AGSZ[s] + (r - AGCUM[s])


# ======================================================================
# host-side preprocessing
# ======================================================================

def _wrap16(idx_flat):
    """dma_gather index layout: idx i at [p, s] with p=i%16 (replicated across
    the 8 groups of 16 partitions), s=i//16."""
    n = idx_flat.size
    assert n % 16 == 0
    a = idx_flat.reshape(n // 16, 16).T.astype(np.int16)   # [16, n/16]
    return np.tile(a, (8, 1))                              # [128, n/16]


def preprocess(x, edge_index, batch):
    """Returns per-core host data + shared static structure.

    Slot layout: partition p = dst-local node, round t = edge slot.
    Rounds are grouped into chunks; each chunk has a shared (across cores)
    base B so that every src index in the chunk fits the int16 window
    [B, B+32768) of the dma_gather. Per-lane edge lists are sorted by src
    and a greedy walk assigns edges to rounds, starting a new chunk when
    any lane's next edge falls outside the current window.
    """
    src0 = edge_index[0].astype(np.int64)
    dst0 = edge_index[1].astype(np.int64)
    loop = np.arange(N, dtype=np.int64)
    src = np.concatenate([src0, loop])
    dst = np.concatenate([dst0, loop])

    deg = np.bincount(dst, minlength=N)

    # new node ids: core = old // NLOC ; within core sort by degree (stable)
    newid = np.empty(N, dtype=np.int64)    # dst id: c*NLOCP + rank
    tabid = np.empty(N, dtype=np.int64)    # gather-table row (chunk-major)
    perm_per_core = []    # old local order for each core (newid -> old)
    ranks = np.arange(NLOC)
    schunk = np.searchsorted(AGCUM, ranks, side="right") - 1
    trow = np.asarray(AGBASE)[schunk] * 1 + (ranks - AGCUM[schunk])
    for c in range(NCOR):
        lo, hi = c * NLOC, (c + 1) * NLOC
        order = np.argsort(deg[lo:hi], kind="stable")      # ascending degree
        perm_per_core.append(order + lo)                   # new local i -> old global
        newid[lo + order] = c * NLOCP + ranks
        tabid[lo + order] = trow + c * np.asarray(AGSZ)[schunk]
    src_n = tabid[src]
    dst_n = newid[dst]

    core_of = dst_n // NLOCP
    dloc = dst_n % NLOCP
    blk = dloc // PBLK
    p = dloc % PBLK

    # sort edges by (block, core, p, src): per-lane sorted src lists
    lane = (blk * NCOR + core_of) * PBLK + p       # block-major lane id
    order = np.lexsort((src_n, lane))
    lane_s = lane[order]
    src_s = src_n[order]
    NLANE = NCOR * PBLK
    cnt = np.bincount(lane, minlength=NBLK * NLANE)
    lane_start = np.r_[0, np.cumsum(cnt)]          # into src_s

    chunks = []        # per block: list of (r0, r1, B)
    T = np.zeros(NBLK, dtype=np.int64)
    slot_rel = []      # per block: [NLANE, T_b] int16 relative idx
    slot_msk = []      # per block: [NLANE, T_b] bool (real edge)
    for b in range(NBLK):
        base0 = b * NLANE
        starts = lane_start[base0:base0 + NLANE].copy()
        ends = lane_start[base0 + 1:base0 + NLANE + 1]
        rel_cols = []
        msk_cols = []
        bchunks = []
        Tb = 0
        while np.any(starts < ends):
            active = starts < ends
            B = int(src_s[starts[active]].min())
            B = min(B, NTAB - SPLIT)
            r0 = Tb
            for _ in range(GCH):
                active = starts < ends
                if not np.any(active):
                    break
                nxt = np.where(active, src_s[np.minimum(starts, len(src_s) - 1)],
                               np.int64(1) << 40)
                place = active & (nxt < B + SPLIT)
                rel = np.where(place, nxt - B, 0).astype(np.int16)
                rel_cols.append(rel)
                msk_cols.append(place)
                starts = starts + place
                Tb += 1
                # keep extending the chunk unless too many lanes stalled
                if (active & ~place).sum() > 0.125 * active.sum():
                    break
            bchunks.append((r0, Tb, B))
        chunks.append(bchunks)
        T[b] = Tb
        slot_rel.append(np.stack(rel_cols, axis=1) if Tb else
                        np.zeros((NLANE, 0), np.int16))
        slot_msk.append(np.stack(msk_cols, axis=1) if Tb else
                        np.zeros((NLANE, 0), bool))

    off = np.r_[0, np.cumsum(T)]               # col offset per block
    TOT = int(off[-1])

    # slot tables per core
    idx_cols = 8 * TOT  # wrapped int16 cols
    idx_all = np.zeros((NCOR, PBLK, idx_cols), dtype=np.int16)
    maskmul = np.zeros((NCOR, PBLK, TOT), dtype=F32)
    for b in range(NBLK):
        o = int(off[b])
        Tb = int(T[b])
        rel_b = slot_rel[b].reshape(NCOR, PBLK, Tb)
        msk_b = slot_msk[b].reshape(NCOR, PBLK, Tb)
        for c in range(NCOR):
            maskmul[c, :, o:o + Tb] = msk_b[c]
            # flat[i] for i = (t-0)*PBLK + p  ->  [Tb*PBLK]
            flat = rel_b[c].T.reshape(-1)
            idx_all[c, :, o * 8:(o + Tb) * 8] = _wrap16(flat)

    # batch / pooling metadata in new order
    counts = np.bincount(batch.astype(np.int64), minlength=B).astype(F32)
    counts = np.maximum(counts, 1.0)
    batchcol = np.zeros((NCOR, PBLK, NBLK), dtype=F32)
    invcnt = np.zeros((NCOR, PBLK, NBLK), dtype=F32)
    xT0 = np.zeros((NCOR, D, NLOCP), dtype=F32)
    for c in range(NCOR):
        old = perm_per_core[c]                    # [NLOC] old global ids
        bt = batch[old].astype(np.int64)          # [NLOC]
        bc = np.zeros(NLOCP, dtype=F32)
        ic = np.zeros(NLOCP, dtype=F32)
        bc[:NLOC] = bt
        ic[:NLOC] = 1.0 / counts[bt]
        batchcol[c] = bc.reshape(NBLK, PBLK).T
        invcnt[c] = ic.reshape(NBLK, PBLK).T
        xT0[c, :, :NLOC] = x[old].T

    static = dict(T=T, off=off, TOT=TOT, chunks=chunks)
    percore = dict(idx_all=idx_all, maskmul=maskmul, batchcol=batchcol,
                   invcnt=invcnt, xT0=xT0)
    return static, percore


def make_waug(W, a_s, a_d):
    cin = W.shape[0]
    als = np.stack([W[:, h * C:(h + 1) * C] @ a_s[h] for h in range(H)], axis=1)
    ald = np.stack([W[:, h * C:(h + 1) * C] @ a_d[h] for h in range(H)], axis=1)
    return np.concatenate([W, als, ald], axis=1).astype(F32)


# ======================================================================
# bass program
# ======================================================================

def build_program(static):
    import os
    import concourse.bacc as bacc
    import concourse.bass as bass
    import concourse.mybir as mybir
    import concourse.tile as tile
    from concourse.masks import make_identity
    from concourse.library_config import mlp

    f32 = mybir.dt.float32
    AFT = mybir.ActivationFunctionType
    ALU = mybir.AluOpType
    T, off, TOT, chunks = (static[k] for k in ("T", "off", "TOT", "chunks"))
    TMAX = int(T.max())

    nq = int(os.environ.get("GAT_NQ", "2"))
    nc = bacc.Bacc(None, target_bir_lowering=False, num_devices=NCOR,
                   num_swdge_queues=nq)

    # ---- I/O ----
    bf16 = mybir.dt.bfloat16
    xT0_d = nc.dram_tensor("xT0", [D, NLOCP], bf16, kind="ExternalInput")
    w_d = {}
    for ell, cin in ((0, D), (1, HC), (2, HC)):
        w_d[ell] = nc.dram_tensor(f"Waug{ell}", [cin, WCOL], bf16, kind="ExternalInput")
    brep_d = {0: nc.dram_tensor("b0rep", [PBLK, HC], f32, kind="ExternalInput"),
              1: nc.dram_tensor("b1rep", [PBLK, HC], f32, kind="ExternalInput"),
              2: nc.dram_tensor("b2rep", [PBLK, C], f32, kind="ExternalInput")}
    idx_d = nc.dram_tensor("idx_all", [PBLK, 8 * TOT], mybir.dt.int16, kind="ExternalInput")
    msk_d = nc.dram_tensor("maskmul", [PBLK, TOT], f32, kind="ExternalInput")
    bcol_d = nc.dram_tensor("batchcol", [PBLK, NBLK], f32, kind="ExternalInput")
    icnt_d = nc.dram_tensor("invcnt", [PBLK, NBLK], f32, kind="ExternalInput")
    iota64_d = nc.dram_tensor("iota64", [PBLK, B], f32, kind="ExternalInput")
    pw1_d = nc.dram_tensor("pW1", [C, C // 2], f32, kind="ExternalInput")
    pb1_d = nc.dram_tensor("pb1", [C // 2, 1], f32, kind="ExternalInput")
    pw2_d = nc.dram_tensor("pW2", [C // 2, OUT], f32, kind="ExternalInput")
    pb2_d = nc.dram_tensor("pb2", [OUT, 1], f32, kind="ExternalInput")
    out_d = nc.dram_tensor("out_t", [OUT, B], f32, kind="ExternalOutput")

    # ---- internals ----
    h_loc = [nc.dram_tensor(f"h_loc{l}", [NLOCP, RB], bf16) for l in range(3)]
    ag = [nc.dram_tensor(f"ag{l}", [NTAB, RB], bf16, addr_space="Shared")
          for l in range(3)]
    pool_in = nc.dram_tensor("pool_in", [C, B], f32)
    pool_out = nc.dram_tensor("pool_out", [C, B], f32, addr_space="Shared")

    groups = [list(range(NCOR))]
    agb = AGB    # AllGather chunk boundaries (blocks)

    with tile.TileContext(nc) as tc:
        with tc.tile_pool(name="const", bufs=1) as cp, \
             tc.tile_pool(name="meta", bufs=1) as mp, \
             tc.tile_pool(name="gbuf", bufs=2) as gp, \
             tc.tile_pool(name="work", bufs=3) as wp, \
             tc.tile_pool(name="dense", bufs=3) as dp, \
             tc.tile_pool(name="xt", bufs=4) as xtp, \
             tc.tile_pool(name="psum_m", bufs=2, space="PSUM") as pm, \
             tc.tile_pool(name="psum_d", bufs=2, space="PSUM") as pd, \
             tc.tile_pool(name="psum_t", bufs=2, space="PSUM") as pt, \
             tc.tile_pool(name="psum_g", bufs=1, space="PSUM") as pg:

            nc.gpsimd.load_library(mlp)

            ident = cp.tile([PBLK, PBLK], dtype=f32)
            make_identity(nc, ident[:])
            identb = cp.tile([PBLK, PBLK], dtype=bf16)
            nc.vector.tensor_copy(out=identb[:], in_=ident[:])

            idx_sb = mp.tile([PBLK, 8 * TOT], dtype=mybir.dt.int16)
            nc.sync.dma_start(out=idx_sb[:], in_=idx_d[:])
            msk_sb = mp.tile([PBLK, TOT], dtype=f32)
            nc.sync.dma_start(out=msk_sb[:], in_=msk_d[:])
            bcol_sb = cp.tile([PBLK, NBLK], dtype=f32)
            nc.sync.dma_start(out=bcol_sb[:], in_=bcol_d[:])
            icnt_sb = cp.tile([PBLK, NBLK], dtype=f32)
            nc.sync.dma_start(out=icnt_sb[:], in_=icnt_d[:])
            iota_sb = cp.tile([PBLK, B], dtype=f32)
            nc.sync.dma_start(out=iota_sb[:], in_=iota64_d[:])

            # weights + biases resident in SBUF
            wsb = {}
            for ell, cin in ((0, D), (1, HC), (2, HC)):
                wsb[ell] = []
                for ccn in range(cin // PBLK):
                    wt = cp.tile([PBLK, WCOL], dtype=bf16,
                                 name=f"wsb{ell}_{ccn}", tag=f"wsb{ell}_{ccn}")
                    nc.sync.dma_start(
                        out=wt[:], in_=w_d[ell][ccn * PBLK:(ccn + 1) * PBLK, :])
                    wsb[ell].append(wt)
            brep = {}
            for ell in range(3):
                brep[ell] = cp.tile([PBLK, HC if ell < 2 else C], dtype=f32,
                                    name=f"brep{ell}", tag=f"brep{ell}")
                nc.sync.dma_start(out=brep[ell][:], in_=brep_d[ell][:])

            pool_ps = pg.tile([C, B], dtype=f32, space="PSUM")

            def fire_ag(ell, s):
                r0, r1 = int(AGCUM[s]), int(AGCUM[s + 1])
                nc.gpsimd.collective_compute(
                    "AllGather", mybir.AluOpType.bypass, replica_groups=groups,
                    ins=[h_loc[ell][r0:r1, :]],
                    outs=[ag[ell][AGBASE[s]:AGBASE[s + 1], :]])

            def dense_block(ell, nt, lhsT_tiles):
                """h = x @ Waug for node block nt; writes h_loc[ell] rows."""
                ph = pd.tile([PBLK, WCOL], dtype=f32, space="PSUM", tag="ph")
                nch = len(lhsT_tiles)
                for ccn, lt in enumerate(lhsT_tiles):
                    nc.tensor.matmul(ph[:], lhsT=lt[:], rhs=wsb[ell][ccn][:],
                                     start=(ccn == 0), stop=(ccn == nch - 1))
                hsb = dp.tile([PBLK, RB], dtype=bf16, tag="hsb")
                nc.vector.memset(hsb[:, HC + 16:], 0.0)
                nc.scalar.activation(hsb[:, 0:HC], ph[:, 0:HC], AFT.Copy)
                nc.vector.tensor_copy(
                    out=hsb[:, HC:HC + 16].bitcast(f32),
                    in_=ph[:, HC:HC + 8])
                nc.sync.dma_start(
                    out=h_loc[ell][nt * PBLK:(nt + 1) * PBLK, :], in_=hsb[:])

            # ---------- layer 0 dense (+ chunked AllGather) ----------
            ags = 0
            for nt in range(NBLK):
                lw = dp.tile([PBLK, PBLK], dtype=bf16, tag="lw")
                nc.sync.dma_start(
                    out=lw[:], in_=xT0_d[:, nt * PBLK:(nt + 1) * PBLK])
                dense_block(0, nt, [lw])
                if nt + 1 == agb[ags + 1]:
                    fire_ag(0, ags)
                    ags += 1

            # ---------- edge phases (dense of next layer interleaved) ----------
            gq = 0
            for ell in range(3):
                ags = 0
                for b in range(NBLK):
                    Tb = int(T[b])
                    ob = int(off[b])
                    G = gp.tile([PBLK, TMAX, RB], dtype=bf16, tag="G")
                    for r0, r1, base in chunks[b]:
                        nidx = (r1 - r0) * PBLK
                        src_view = ag[ell][base:base + SPLIT, :]
                        nc.gpsimd.dma_gather(
                            G[:, r0:r1, :], src_view,
                            idx_sb[:, (ob + r0) * 8:(ob + r1) * 8],
                            nidx, nidx, RB, queue_num=gq % nq)
                        gq += 1
                    ald8 = wp.tile([PBLK, 8], dtype=bf16, tag="ald8")
                    nc.sync.dma_start(
                        out=ald8[:],
                        in_=h_loc[ell][b * PBLK:(b + 1) * PBLK, HC + 8:HC + 16])
                    ald = ald8[:].bitcast(f32)
                    # X = exp(lrelu(als + ald)); Xb = X * mask -> G[:, :, 256:260]
                    X = wp.tile([PBLK, TMAX, 4], dtype=f32, tag="X")
                    nc.vector.tensor_tensor(
                        out=X[:, 0:Tb, :],
                        in0=G[:, 0:Tb, HC:HC + 8].bitcast(f32)[:, :, 0:4],
                        in1=ald[:, None, :].to_broadcast([PBLK, Tb, 4]),
                        op=ALU.add)
                    Xs = wp.tile([PBLK, TMAX, 4], dtype=f32, tag="Xs")
                    nc.vector.tensor_scalar(out=Xs[:, 0:Tb, :], in0=X[:, 0:Tb, :],
                                            scalar1=SLOPE, scalar2=None,
                                            op0=ALU.mult)
                    nc.vector.tensor_tensor(out=X[:, 0:Tb, :], in0=X[:, 0:Tb, :],
                                            in1=Xs[:, 0:Tb, :], op=ALU.max)
                    nc.scalar.activation(X[:, 0:Tb, :], X[:, 0:Tb, :], AFT.Exp)
                    nc.vector.tensor_tensor(
                        out=G[:, 0:Tb, HC:HC + 4],
                        in0=X[:, 0:Tb, :],
                        in1=msk_sb[:, ob:ob + Tb, None].to_broadcast([PBLK, Tb, 4]),
                        op=ALU.mult)
                    # scale h cols per head in one 4-D broadcast op
                    nc.vector.tensor_tensor(
                        out=G[:, 0:Tb, 0:HC].rearrange("p t (h c) -> p t h c", h=H),
                        in0=G[:, 0:Tb, 0:HC].rearrange("p t (h c) -> p t h c", h=H),
                        in1=G[:, 0:Tb, HC:HC + 4, None].to_broadcast(
                            [PBLK, Tb, H, C]),
                        op=ALU.mult)
                    # accumulate [num | den] over rounds into PSUM
                    M = pm.tile([PBLK, HC + 4], dtype=f32, space="PSUM", tag="M")
                    for t in range(Tb):
                        nc.tensor.matmul(M[:], lhsT=identb[:],
                                         rhs=G[:, t, 0:HC + 4],
                                         start=(t == 0), stop=(t == Tb - 1))
                    # normalize: oh = M[:, 0:HC] / (den + eps)
                    den = wp.tile([PBLK, 4], dtype=f32, tag="den")
                    nc.vector.tensor_scalar(out=den[:], in0=M[:, HC:HC + 4],
                                            scalar1=1e-16, scalar2=None,
                                            op0=ALU.add)
                    rec = wp.tile([PBLK, 4], dtype=f32, tag="rec")
                    nc.vector.reciprocal(rec[:], den[:])
                    oh = wp.tile([PBLK, HC], dtype=f32, tag="oh")
                    nc.vector.tensor_tensor(
                        out=oh[:].rearrange("p (h c) -> p h c", h=H),
                        in0=M[:, 0:HC].rearrange("p (h c) -> p h c", h=H),
                        in1=rec[:, :, None].to_broadcast([PBLK, H, C]),
                        op=ALU.mult)
                    if ell < 2:
                        # oh = elu(oh + bias)
                        nc.vector.tensor_tensor(out=oh[:], in0=oh[:],
                                                in1=brep[ell][:], op=ALU.add)
                        mn = wp.tile([PBLK, HC], dtype=f32, tag="mn")
                        nc.vector.tensor_scalar(out=mn[:], in0=oh[:], scalar1=0.0,
                                                scalar2=None, op0=ALU.min)
                        ex = wp.tile([PBLK, HC], dtype=f32, tag="ex")
                        nc.scalar.activation(ex[:], mn[:], AFT.Exp)
                        nc.vector.tensor_scalar(out=ex[:], in0=ex[:], scalar1=-1.0,
                                                scalar2=None, op0=ALU.add)
                        nc.vector.tensor_tensor(out=oh[:], in0=oh[:], in1=ex[:],
                                                op=ALU.max)
                        # transpose to [ch, node] bf16 tiles; dense of next layer
                        ohb = wp.tile([PBLK, HC], dtype=bf16, tag="ohb")
                        nc.vector.tensor_copy(out=ohb[:], in_=oh[:])
                        tps_tiles = []
                        for ccn in range(2):
                            tp = pt.tile([PBLK, PBLK], dtype=bf16, space="PSUM",
                                         tag="tp")
                            nc.tensor.transpose(
                                tp[:], ohb[:, ccn * PBLK:(ccn + 1) * PBLK],
                                identb[:])
                            tps = xtp.tile([PBLK, PBLK], dtype=bf16, tag="tps")
                            nc.scalar.activation(tps[:], tp[:], AFT.Copy)
                            tps_tiles.append(tps)
                        dense_block(ell + 1, b, tps_tiles)
                        if b + 1 == agb[ags + 1]:
                            fire_ag(ell + 1, ags)
                            ags += 1
                    else:
                        # mean over heads + bias, then pooling contribution
                        o64 = wp.tile([PBLK, C], dtype=f32, tag="o64")
                        nc.vector.tensor_tensor(out=o64[:], in0=oh[:, 0:C],
                                                in1=oh[:, C:2 * C], op=ALU.add)
                        nc.vector.tensor_tensor(out=o64[:], in0=o64[:],
                                                in1=oh[:, 2 * C:3 * C], op=ALU.add)
                        nc.vector.tensor_tensor(out=o64[:], in0=o64[:],
                                                in1=oh[:, 3 * C:4 * C], op=ALU.add)
                        nc.vector.tensor_scalar(out=o64[:], in0=o64[:],
                                                scalar1=0.25, scalar2=None,
                                                op0=ALU.mult)
                        nc.vector.tensor_tensor(out=o64[:], in0=o64[:],
                                                in1=brep[2][:], op=ALU.add)
                        # scale by 1/count, build batch one-hot, accumulate
                        nc.vector.tensor_tensor(
                            out=o64[:], in0=o64[:],
                            in1=icnt_sb[:, b:b + 1].to_broadcast([PBLK, C]),
                            op=ALU.mult)
                        bh = wp.tile([PBLK, B], dtype=f32, tag="bh")
                        nc.vector.tensor_tensor(
                            out=bh[:],
                            in0=bcol_sb[:, b:b + 1].to_broadcast([PBLK, B]),
                            in1=iota_sb[:], op=ALU.is_equal)
                        nc.tensor.matmul(pool_ps[:], lhsT=o64[:], rhs=bh[:],
                                         start=(b == 0), stop=(b == NBLK - 1))

            # ---------- pooled AllReduce + MLP ----------
            pool_sb = wp.tile([C, B], dtype=f32, tag="pool_sb")
            nc.scalar.activation(pool_sb[:], pool_ps[:], AFT.Copy)
            nc.sync.dma_start(out=pool_in[:], in_=pool_sb[:])
            nc.gpsimd.collective_compute(
                "AllReduce", mybir.AluOpType.add, replica_groups=groups,
                ins=[pool_in[:]], outs=[pool_out[:]])
            pooled = wp.tile([C, B], dtype=f32, tag="pooled")
            nc.sync.dma_start(out=pooled[:], in_=pool_out[:])

            pw1 = cp.tile([C, C // 2], dtype=f32)
            nc.sync.dma_start(out=pw1[:], in_=pw1_d[:])
            pb1 = cp.tile([C // 2, 1], dtype=f32)
            nc.sync.dma_start(out=pb1[:], in_=pb1_d[:])
            pw2 = cp.tile([C // 2, OUT], dtype=f32)
            nc.sync.dma_start(out=pw2[:], in_=pw2_d[:])
            pb2 = cp.tile([OUT, 1], dtype=f32)
            nc.sync.dma_start(out=pb2[:], in_=pb2_d[:])

            z1p = pt.tile([C // 2, B], dtype=f32, space="PSUM", tag="tp")
            nc.tensor.matmul(z1p[:], lhsT=pw1[:], rhs=pooled[:], start=True, stop=True)
            z1 = wp.tile([C // 2, B], dtype=f32, tag="z1")
            nc.scalar.activation(z1[:], z1p[:], AFT.Relu, bias=pb1[:, 0:1])
            z2p = pt.tile([OUT, B], dtype=f32, space="PSUM", tag="tp")
            nc.tensor.matmul(z2p[:], lhsT=pw2[:], rhs=z1[:], start=True, stop=True)
            z2 = wp.tile([OUT, B], dtype=f32, tag="z2")
            nc.vector.tensor_scalar(out=z2[:], in0=z2p[:], scalar1=pb2[:, 0:1],
                                    scalar2=None, op0=ALU.add)
            nc.sync.dma_start(out=out_d[:], in_=z2[:])

    nc.compile()
    return nc


# ======================================================================
# entry point
# ======================================================================

def kernel(x, edge_index, batch, W0, b0, as0, ad0, W1, b1, as1, ad1,
           W2, b2, as2, ad2, pW1, pb1, pW2, pb2):
    x = np.asarray(x, dtype=F32)
    edge_index = np.asarray(edge_index)
    batch = np.asarray(batch)

    static, percore = preprocess(x, edge_index, batch)

    waug = {0: make_waug(np.asarray(W0, F32), np.asarray(as0, F32), np.asarray(ad0, F32)),
            1: make_waug(np.asarray(W1, F32), np.asarray(as1, F32), np.asarray(ad1, F32)),
            2: make_waug(np.asarray(W2, F32), np.asarray(as2, F32), np.asarray(ad2, F32))}
    b0r = np.broadcast_to(np.asarray(b0, F32), (PBLK, HC)).copy()
    b1r = np.broadcast_to(np.asarray(b1, F32), (PBLK, HC)).copy()
    b2r = np.broadcast_to(np.asarray(b2, F32), (PBLK, C)).copy()
    iota64 = np.broadcast_to(np.arange(B, dtype=F32), (PBLK, B)).copy()

    nc = build_program(static)

    import ml_dtypes
    bf = ml_dtypes.bfloat16
    waug = {k: v.astype(bf) for k, v in waug.items()}
    from concourse.bass_utils import run_bass_kernel_spmd
    in_maps = []
    for c in range(NCOR):
        in_maps.append(dict(
            xT0=percore["xT0"][c].astype(bf),
            Waug0=waug[0], Waug1=waug[1], Waug2=waug[2],
            b0rep=b0r, b1rep=b1r, b2rep=b2r,
            idx_all=percore["idx_all"][c],
            maskmul=percore["maskmul"][c],
            batchcol=percore["batchcol"][c],
            invcnt=percore["invcnt"][c],
            iota64=iota64,
            pW1=np.asarray(pW1, F32), pb1=np.asarray(pb1, F32).reshape(-1, 1),
            pW2=np.asarray(pW2, F32), pb2=np.asarray(pb2, F32).reshape(-1, 1),
        ))
    import os as _os
    trace = _os.environ.get("GAT_TRACE", "0") == "1"
    kw = {}
    if trace:
        kw = dict(trace=True, tmpdir=_os.environ.get("GAT_TRACE_DIR") or None)
    res = run_bass_kernel_spmd(nc, in_maps, list(range(NCOR)), **kw)
    if trace:
        print(f"HW exec time: {res.exec_time_ns} ns")
    out_t = res.results[0]["out_t"]            # [OUT, B]
    return np.ascontiguousarray(out_t.T).astype(F32)



# revision 25
# speedup vs baseline: 1.3989x; 1.3989x over previous
"""GAT (3-layer, 4-head) forward on 8 Trainium2 NeuronCores.

Strategy: partition nodes by destination across 8 cores (graph-parallel),
renumber nodes so each core's shard is degree-sorted, route edges to the
dst-owning core in a degree-slot layout (slot = (dst_local partition, round)).
Segment softmax + scatter then reduce to plain PSUM accumulation of
exp-scaled gathered rows via an identity matmul. Attention coefficients
al_src/al_dst are folded into the dense-phase weight matrix as extra output
columns, so one bulk dma_gather per block fetches everything per edge.
Halo exchange of h rows via AllGather each layer.
"""
import sys
sys.path.insert(0, "/opt/trn_rl_repo")
import numpy as np

# ---- problem constants (hardcoded per contest contract) ----
N = 50000
E = 800000
D = 128
H = 4
C = 64
HC = 256
B = 64
OUT = 10
SLOPE = 0.2

NCOR = 8
NLOC = N // NCOR          # 6250
PBLK = 128
NBLK = (NLOC + PBLK - 1) // PBLK      # 49
NLOCP = NBLK * PBLK       # 6272 padded local nodes
NTAB = NCOR * NLOCP       # 50176 global padded table rows
ROW = 320                 # legacy f32 row (unused in bf16 mode)
WCOL = 264                # Waug output cols: 256 h | 4 als | 4 ald
RB = 384                  # bf16 cols per h row (768B): 256 h | 16 al-f32-bits | pad
SPLIT = 32768             # int16 gather index limit (window size)
GCH = 8                   # dma_gather caps at 1024 indices = 8 rounds
F32 = np.float32

# AllGather row chunks (in blocks). The shared gather table is laid out
# chunk-major ([all cores chunk0 | all cores chunk1 | ...]) so each chunked
# AllGather writes one contiguous slice.
AGS = 4
AGB = [0] + [round((s + 1) * NBLK / AGS) for s in range(AGS)]
AGSZ = [(AGB[s + 1] - AGB[s]) * PBLK for s in range(AGS)]   # rows/core/chunk
AGCUM = np.r_[0, np.cumsum(AGSZ)]                           # per-core offsets
AGBASE = [int(NCOR * AGCUM[s]) for s in range(AGS + 1)]     # table offsets


def table_row(c, r):
    """Global gather-table row for local row r of core c (chunk-major)."""
    s = np.searchsorted(AGCUM, r, side="right") - 1
    return AGBASE[s] + c * AGSZ[s] + (r - AGCUM[s])


# ======================================================================
# host-side preprocessing
# ======================================================================

def _wrap16(idx_flat):
    """dma_gather index layout: idx i at [p, s] with p=i%16 (replicated across
    the 8 groups of 16 partitions), s=i//16."""
    n = idx_flat.size
    assert n % 16 == 0
    a = idx_flat.reshape(n // 16, 16).T.astype(np.int16)   # [16, n/16]
    return np.tile(a, (8, 1))                              # [128, n/16]


def preprocess(x, edge_index, batch):
    """Returns per-core host data + shared static structure.

    Slot layout: partition p = dst-local node, round t = edge slot.
    Rounds are grouped into chunks; each chunk has a shared (across cores)
    base B so that every src index in the chunk fits the int16 window
    [B, B+32768) of the dma_gather. Per-lane edge lists are sorted by src
    and a greedy walk assigns edges to rounds, starting a new chunk when
    any lane's next edge falls outside the current window.
    """
    src0 = edge_index[0].astype(np.int64)
    dst0 = edge_index[1].astype(np.int64)
    loop = np.arange(N, dtype=np.int64)
    src = np.concatenate([src0, loop])
    dst = np.concatenate([dst0, loop])

    deg = np.bincount(dst, minlength=N)

    # new node ids: core = old // NLOC ; within core sort by degree (stable)
    newid = np.empty(N, dtype=np.int64)    # dst id: c*NLOCP + rank
    tabid = np.empty(N, dtype=np.int64)    # gather-table row (chunk-major)
    perm_per_core = []    # old local order for each core (newid -> old)
    ranks = np.arange(NLOC)
    schunk = np.searchsorted(AGCUM, ranks, side="right") - 1
    trow = np.asarray(AGBASE)[schunk] * 1 + (ranks - AGCUM[schunk])
    for c in range(NCOR):
        lo, hi = c * NLOC, (c + 1) * NLOC
        order = np.argsort(deg[lo:hi], kind="stable")      # ascending degree
        perm_per_core.append(order + lo)                   # new local i -> old global
        newid[lo + order] = c * NLOCP + ranks
        tabid[lo + order] = trow + c * np.asarray(AGSZ)[schunk]
    src_n = tabid[src]
    dst_n = newid[dst]

    core_of = dst_n // NLOCP
    dloc = dst_n % NLOCP
    blk = dloc // PBLK
    p = dloc % PBLK

    # sort edges by (block, core, p, src): per-lane sorted src lists
    lane = (blk * NCOR + core_of) * PBLK + p       # block-major lane id
    order = np.lexsort((src_n, lane))
    lane_s = lane[order]
    src_s = src_n[order]
    NLANE = NCOR * PBLK
    cnt = np.bincount(lane, minlength=NBLK * NLANE)
    lane_start = np.r_[0, np.cumsum(cnt)]          # into src_s

    chunks = []        # per block: list of (r0, r1, B)
    T = np.zeros(NBLK, dtype=np.int64)
    slot_rel = []      # per block: [NLANE, T_b] int16 relative idx
    slot_msk = []      # per block: [NLANE, T_b] bool (real edge)
    for b in range(NBLK):
        base0 = b * NLANE
        starts = lane_start[base0:base0 + NLANE].copy()
        ends = lane_start[base0 + 1:base0 + NLANE + 1]
        rel_cols = []
        msk_cols = []
        bchunks = []
        Tb = 0
        while np.any(starts < ends):
            active = starts < ends
            B = int(src_s[starts[active]].min())
            B = min(B, NTAB - SPLIT)
            r0 = Tb
            for _ in range(GCH):
                active = starts < ends
                if not np.any(active):
                    break
                nxt = np.where(active, src_s[np.minimum(starts, len(src_s) - 1)],
                               np.int64(1) << 40)
                place = active & (nxt < B + SPLIT)
                rel = np.where(place, nxt - B, 0).astype(np.int16)
                rel_cols.append(rel)
                msk_cols.append(place)
                starts = starts + place
                Tb += 1
                # any stalled lane -> new window (total rounds dominate cost)
                if np.any(active & ~place):
                    break
            bchunks.append((r0, Tb, B))
        chunks.append(bchunks)
        T[b] = Tb
        slot_rel.append(np.stack(rel_cols, axis=1) if Tb else
                        np.zeros((NLANE, 0), np.int16))
        slot_msk.append(np.stack(msk_cols, axis=1) if Tb else
                        np.zeros((NLANE, 0), bool))

    off = np.r_[0, np.cumsum(T)]               # col offset per block
    TOT = int(off[-1])

    # slot tables per core
    idx_cols = 8 * TOT  # wrapped int16 cols
    idx_all = np.zeros((NCOR, PBLK, idx_cols), dtype=np.int16)
    maskmul = np.zeros((NCOR, PBLK, TOT), dtype=F32)
    for b in range(NBLK):
        o = int(off[b])
        Tb = int(T[b])
        rel_b = slot_rel[b].reshape(NCOR, PBLK, Tb)
        msk_b = slot_msk[b].reshape(NCOR, PBLK, Tb)
        for c in range(NCOR):
            maskmul[c, :, o:o + Tb] = msk_b[c]
            # flat[i] for i = (t-0)*PBLK + p  ->  [Tb*PBLK]
            flat = rel_b[c].T.reshape(-1)
            idx_all[c, :, o * 8:(o + Tb) * 8] = _wrap16(flat)

    # batch / pooling metadata in new order
    counts = np.bincount(batch.astype(np.int64), minlength=B).astype(F32)
    counts = np.maximum(counts, 1.0)
    batchcol = np.zeros((NCOR, PBLK, NBLK), dtype=F32)
    invcnt = np.zeros((NCOR, PBLK, NBLK), dtype=F32)
    xT0 = np.zeros((NCOR, D, NLOCP), dtype=F32)
    for c in range(NCOR):
        old = perm_per_core[c]                    # [NLOC] old global ids
        bt = batch[old].astype(np.int64)          # [NLOC]
        bc = np.zeros(NLOCP, dtype=F32)
        ic = np.zeros(NLOCP, dtype=F32)
        bc[:NLOC] = bt
        ic[:NLOC] = 1.0 / counts[bt]
        batchcol[c] = bc.reshape(NBLK, PBLK).T
        invcnt[c] = ic.reshape(NBLK, PBLK).T
        xT0[c, :, :NLOC] = x[old].T

    static = dict(T=T, off=off, TOT=TOT, chunks=chunks)
    percore = dict(idx_all=idx_all, maskmul=maskmul, batchcol=batchcol,
                   invcnt=invcnt, xT0=xT0)
    return static, percore


def make_waug(W, a_s, a_d):
    cin = W.shape[0]
    als = np.stack([W[:, h * C:(h + 1) * C] @ a_s[h] for h in range(H)], axis=1)
    ald = np.stack([W[:, h * C:(h + 1) * C] @ a_d[h] for h in range(H)], axis=1)
    return np.concatenate([W, als, ald], axis=1).astype(F32)


# ======================================================================
# bass program
# ======================================================================

def build_program(static):
    import os
    import concourse.bacc as bacc
    import concourse.bass as bass
    import concourse.mybir as mybir
    import concourse.tile as tile
    from concourse.masks import make_identity
    from concourse.library_config import mlp

    f32 = mybir.dt.float32
    AFT = mybir.ActivationFunctionType
    ALU = mybir.AluOpType
    T, off, TOT, chunks = (static[k] for k in ("T", "off", "TOT", "chunks"))
    TMAX = int(T.max())

    nq = int(os.environ.get("GAT_NQ", "2"))
    nc = bacc.Bacc(None, target_bir_lowering=False, num_devices=NCOR,
                   num_swdge_queues=nq)

    # ---- I/O ----
    bf16 = mybir.dt.bfloat16
    xT0_d = nc.dram_tensor("xT0", [D, NLOCP], bf16, kind="ExternalInput")
    w_d = {}
    for ell, cin in ((0, D), (1, HC), (2, HC)):
        w_d[ell] = nc.dram_tensor(f"Waug{ell}", [cin, WCOL], bf16, kind="ExternalInput")
    brep_d = {0: nc.dram_tensor("b0rep", [PBLK, HC], f32, kind="ExternalInput"),
              1: nc.dram_tensor("b1rep", [PBLK, HC], f32, kind="ExternalInput"),
              2: nc.dram_tensor("b2rep", [PBLK, C], f32, kind="ExternalInput")}
    idx_d = nc.dram_tensor("idx_all", [PBLK, 8 * TOT], mybir.dt.int16, kind="ExternalInput")
    msk_d = nc.dram_tensor("maskmul", [PBLK, TOT], f32, kind="ExternalInput")
    bcol_d = nc.dram_tensor("batchcol", [PBLK, NBLK], f32, kind="ExternalInput")
    icnt_d = nc.dram_tensor("invcnt", [PBLK, NBLK], f32, kind="ExternalInput")
    iota64_d = nc.dram_tensor("iota64", [PBLK, B], f32, kind="ExternalInput")
    pw1_d = nc.dram_tensor("pW1", [C, C // 2], f32, kind="ExternalInput")
    pb1_d = nc.dram_tensor("pb1", [C // 2, 1], f32, kind="ExternalInput")
    pw2_d = nc.dram_tensor("pW2", [C // 2, OUT], f32, kind="ExternalInput")
    pb2_d = nc.dram_tensor("pb2", [OUT, 1], f32, kind="ExternalInput")
    out_d = nc.dram_tensor("out_t", [OUT, B], f32, kind="ExternalOutput")

    # ---- internals ----
    h_loc = [nc.dram_tensor(f"h_loc{l}", [NLOCP, RB], bf16) for l in range(3)]
    ag = [nc.dram_tensor(f"ag{l}", [NTAB, RB], bf16, addr_space="Shared")
          for l in range(3)]
    pool_in = nc.dram_tensor("pool_in", [C, B], f32)
    pool_out = nc.dram_tensor("pool_out", [C, B], f32, addr_space="Shared")

    groups = [list(range(NCOR))]
    agb = AGB    # AllGather chunk boundaries (blocks)

    with tile.TileContext(nc) as tc:
        with tc.tile_pool(name="const", bufs=1) as cp, \
             tc.tile_pool(name="meta", bufs=1) as mp, \
             tc.tile_pool(name="gbuf", bufs=3) as gp, \
             tc.tile_pool(name="work", bufs=4) as wp, \
             tc.tile_pool(name="dense", bufs=4) as dp, \
             tc.tile_pool(name="xt", bufs=4) as xtp, \
             tc.tile_pool(name="psum_m", bufs=3, space="PSUM") as pm, \
             tc.tile_pool(name="psum_d", bufs=2, space="PSUM") as pd, \
             tc.tile_pool(name="psum_t", bufs=2, space="PSUM") as pt, \
             tc.tile_pool(name="psum_g", bufs=1, space="PSUM") as pg:

            nc.gpsimd.load_library(mlp)

            ident = cp.tile([PBLK, PBLK], dtype=f32)
            make_identity(nc, ident[:])
            identb = cp.tile([PBLK, PBLK], dtype=bf16)
            nc.vector.tensor_copy(out=identb[:], in_=ident[:])

            idx_sb = mp.tile([PBLK, 8 * TOT], dtype=mybir.dt.int16)
            nc.sync.dma_start(out=idx_sb[:], in_=idx_d[:])
            msk_sb = mp.tile([PBLK, TOT], dtype=f32)
            nc.sync.dma_start(out=msk_sb[:], in_=msk_d[:])
            bcol_sb = cp.tile([PBLK, NBLK], dtype=f32)
            nc.sync.dma_start(out=bcol_sb[:], in_=bcol_d[:])
            icnt_sb = cp.tile([PBLK, NBLK], dtype=f32)
            nc.sync.dma_start(out=icnt_sb[:], in_=icnt_d[:])
            iota_sb = cp.tile([PBLK, B], dtype=f32)
            nc.sync.dma_start(out=iota_sb[:], in_=iota64_d[:])

            # weights + biases resident in SBUF
            wsb = {}
            for ell, cin in ((0, D), (1, HC), (2, HC)):
                wsb[ell] = []
                for ccn in range(cin // PBLK):
                    wt = cp.tile([PBLK, WCOL], dtype=bf16,
                                 name=f"wsb{ell}_{ccn}", tag=f"wsb{ell}_{ccn}")
                    nc.sync.dma_start(
                        out=wt[:], in_=w_d[ell][ccn * PBLK:(ccn + 1) * PBLK, :])
                    wsb[ell].append(wt)
            brep = {}
            for ell in range(3):
                brep[ell] = cp.tile([PBLK, HC if ell < 2 else C], dtype=f32,
                                    name=f"brep{ell}", tag=f"brep{ell}")
                nc.sync.dma_start(out=brep[ell][:], in_=brep_d[ell][:])

            pool_ps = pg.tile([C, B], dtype=f32, space="PSUM")

            def fire_ag(ell, s):
                r0, r1 = int(AGCUM[s]), int(AGCUM[s + 1])
                nc.gpsimd.collective_compute(
                    "AllGather", mybir.AluOpType.bypass, replica_groups=groups,
                    ins=[h_loc[ell][r0:r1, :]],
                    outs=[ag[ell][AGBASE[s]:AGBASE[s + 1], :]])

            def dense_block(ell, nt, lhsT_tiles):
                """h = x @ Waug for node block nt; writes h_loc[ell] rows."""
                ph = pd.tile([PBLK, WCOL], dtype=f32, space="PSUM", tag="ph")
                nch = len(lhsT_tiles)
                for ccn, lt in enumerate(lhsT_tiles):
                    nc.tensor.matmul(ph[:], lhsT=lt[:], rhs=wsb[ell][ccn][:],
                                     start=(ccn == 0), stop=(ccn == nch - 1))
                hsb = dp.tile([PBLK, RB], dtype=bf16, tag="hsb")
                nc.vector.memset(hsb[:, HC + 16:], 0.0)
                nc.scalar.activation(hsb[:, 0:HC], ph[:, 0:HC], AFT.Copy)
                nc.vector.tensor_copy(
                    out=hsb[:, HC:HC + 16].bitcast(f32),
                    in_=ph[:, HC:HC + 8])
                nc.sync.dma_start(
                    out=h_loc[ell][nt * PBLK:(nt + 1) * PBLK, :], in_=hsb[:])

            # ---------- layer 0 dense (+ chunked AllGather) ----------
            ags = 0
            for nt in range(NBLK):
                lw = dp.tile([PBLK, PBLK], dtype=bf16, tag="lw")
                nc.sync.dma_start(
                    out=lw[:], in_=xT0_d[:, nt * PBLK:(nt + 1) * PBLK])
                dense_block(0, nt, [lw])
                if nt + 1 == agb[ags + 1]:
                    fire_ag(0, ags)
                    ags += 1

            # ---------- edge phases (dense of next layer interleaved) ----------
            gq = 0
            for ell in range(3):
                ags = 0
                for b in range(NBLK):
                    Tb = int(T[b])
                    ob = int(off[b])
                    G = gp.tile([PBLK, TMAX, RB], dtype=bf16, tag="G")
                    for r0, r1, base in chunks[b]:
                        nidx = (r1 - r0) * PBLK
                        src_view = ag[ell][base:base + SPLIT, :]
                        nc.gpsimd.dma_gather(
                            G[:, r0:r1, :], src_view,
                            idx_sb[:, (ob + r0) * 8:(ob + r1) * 8],
                            nidx, nidx, RB, queue_num=gq % nq)
                        gq += 1
                    ald8 = wp.tile([PBLK, 8], dtype=bf16, tag="ald8")
                    nc.sync.dma_start(
                        out=ald8[:],
                        in_=h_loc[ell][b * PBLK:(b + 1) * PBLK, HC + 8:HC + 16])
                    ald = ald8[:].bitcast(f32)
                    # X = exp(lrelu(als + ald)); Xb = X * mask -> G[:, :, 256:260]
                    X = wp.tile([PBLK, TMAX, 4], dtype=f32, tag="X")
                    nc.vector.tensor_tensor(
                        out=X[:, 0:Tb, :],
                        in0=G[:, 0:Tb, HC:HC + 8].bitcast(f32)[:, :, 0:4],
                        in1=ald[:, None, :].to_broadcast([PBLK, Tb, 4]),
                        op=ALU.add)
                    Xs = wp.tile([PBLK, TMAX, 4], dtype=f32, tag="Xs")
                    nc.vector.tensor_scalar(out=Xs[:, 0:Tb, :], in0=X[:, 0:Tb, :],
                                            scalar1=SLOPE, scalar2=None,
                                            op0=ALU.mult)
                    nc.vector.tensor_tensor(out=X[:, 0:Tb, :], in0=X[:, 0:Tb, :],
                                            in1=Xs[:, 0:Tb, :], op=ALU.max)
                    nc.scalar.activation(X[:, 0:Tb, :], X[:, 0:Tb, :], AFT.Exp)
                    nc.vector.tensor_tensor(
                        out=G[:, 0:Tb, HC:HC + 4],
                        in0=X[:, 0:Tb, :],
                        in1=msk_sb[:, ob:ob + Tb, None].to_broadcast([PBLK, Tb, 4]),
                        op=ALU.mult)
                    # scale h cols per head in one 4-D broadcast op
                    nc.vector.tensor_tensor(
                        out=G[:, 0:Tb, 0:HC].rearrange("p t (h c) -> p t h c", h=H),
                        in0=G[:, 0:Tb, 0:HC].rearrange("p t (h c) -> p t h c", h=H),
                        in1=G[:, 0:Tb, HC:HC + 4, None].to_broadcast(
                            [PBLK, Tb, H, C]),
                        op=ALU.mult)
                    # accumulate [num | den] over rounds into PSUM
                    M = pm.tile([PBLK, HC + 4], dtype=f32, space="PSUM", tag="M")
                    for t in range(Tb):
                        nc.tensor.matmul(M[:], lhsT=identb[:],
                                         rhs=G[:, t, 0:HC + 4],
                                         start=(t == 0), stop=(t == Tb - 1))
                    # normalize: oh = M[:, 0:HC] / (den + eps)
                    den = wp.tile([PBLK, 4], dtype=f32, tag="den")
                    nc.vector.tensor_scalar(out=den[:], in0=M[:, HC:HC + 4],
                                            scalar1=1e-16, scalar2=None,
                                            op0=ALU.add)
                    rec = wp.tile([PBLK, 4], dtype=f32, tag="rec")
                    nc.vector.reciprocal(rec[:], den[:])
                    oh = wp.tile([PBLK, HC], dtype=f32, tag="oh")
                    nc.vector.tensor_tensor(
                        out=oh[:].rearrange("p (h c) -> p h c", h=H),
                        in0=M[:, 0:HC].rearrange("p (h c) -> p h c", h=H),
                        in1=rec[:, :, None].to_broadcast([PBLK, H, C]),
                        op=ALU.mult)
                    if ell < 2:
                        # oh = elu(oh + bias)
                        nc.vector.tensor_tensor(out=oh[:], in0=oh[:],
                                                in1=brep[ell][:], op=ALU.add)
                        mn = wp.tile([PBLK, HC], dtype=f32, tag="mn")
                        nc.vector.tensor_scalar(out=mn[:], in0=oh[:], scalar1=0.0,
                                                scalar2=None, op0=ALU.min)
                        ex = wp.tile([PBLK, HC], dtype=f32, tag="ex")
                        nc.scalar.activation(ex[:], mn[:], AFT.Exp)
                        nc.vector.tensor_scalar(out=ex[:], in0=ex[:], scalar1=-1.0,
                                                scalar2=None, op0=ALU.add)
                        nc.vector.tensor_tensor(out=oh[:], in0=oh[:], in1=ex[:],
                                                op=ALU.max)
                        # transpose to [ch, node] bf16 tiles; dense of next layer
                        ohb = wp.tile([PBLK, HC], dtype=bf16, tag="ohb")
                        nc.vector.tensor_copy(out=ohb[:], in_=oh[:])
                        tps_tiles = []
                        for ccn in range(2):
                            tp = pt.tile([PBLK, PBLK], dtype=bf16, space="PSUM",
                                         tag="tp")
                            nc.tensor.transpose(
                                tp[:], ohb[:, ccn * PBLK:(ccn + 1) * PBLK],
                                identb[:])
                            tps = xtp.tile([PBLK, PBLK], dtype=bf16, tag="tps")
                            nc.scalar.activation(tps[:], tp[:], AFT.Copy)
                            tps_tiles.append(tps)
                        dense_block(ell + 1, b, tps_tiles)
                        if b + 1 == agb[ags + 1]:
                            fire_ag(ell + 1, ags)
                            ags += 1
                    else:
                        # mean over heads + bias, then pooling contribution
                        o64 = wp.tile([PBLK, C], dtype=f32, tag="o64")
                        nc.vector.tensor_tensor(out=o64[:], in0=oh[:, 0:C],
                                                in1=oh[:, C:2 * C], op=ALU.add)
                        nc.vector.tensor_tensor(out=o64[:], in0=o64[:],
                                                in1=oh[:, 2 * C:3 * C], op=ALU.add)
                        nc.vector.tensor_tensor(out=o64[:], in0=o64[:],
                                                in1=oh[:, 3 * C:4 * C], op=ALU.add)
                        nc.vector.tensor_scalar(out=o64[:], in0=o64[:],
                                                scalar1=0.25, scalar2=None,
                                                op0=ALU.mult)
                        nc.vector.tensor_tensor(out=o64[:], in0=o64[:],
                                                in1=brep[2][:], op=ALU.add)
                        # scale by 1/count, build batch one-hot, accumulate
                        nc.vector.tensor_tensor(
                            out=o64[:], in0=o64[:],
                            in1=icnt_sb[:, b:b + 1].to_broadcast([PBLK, C]),
                            op=ALU.mult)
                        bh = wp.tile([PBLK, B], dtype=f32, tag="bh")
                        nc.vector.tensor_tensor(
                            out=bh[:],
                            in0=bcol_sb[:, b:b + 1].to_broadcast([PBLK, B]),
                            in1=iota_sb[:], op=ALU.is_equal)
                        nc.tensor.matmul(pool_ps[:], lhsT=o64[:], rhs=bh[:],
                                         start=(b == 0), stop=(b == NBLK - 1))

            # ---------- pooled AllReduce + MLP ----------
            pool_sb = wp.tile([C, B], dtype=f32, tag="pool_sb")
            nc.scalar.activation(pool_sb[:], pool_ps[:], AFT.Copy)
            nc.sync.dma_start(out=pool_in[:], in_=pool_sb[:])
            nc.gpsimd.collective_compute(
                "AllReduce", mybir.AluOpType.add, replica_groups=groups,
                ins=[pool_in[:]], outs=[pool_out[:]])
            pooled = wp.tile([C, B], dtype=f32, tag="pooled")
            nc.sync.dma_start(out=pooled[:], in_=pool_out[:])

            pw1 = cp.tile([C, C // 2], dtype=f32)
            nc.sync.dma_start(out=pw1[:], in_=pw1_d[:])
            pb1 = cp.tile([C // 2, 1], dtype=f32)
            nc.sync.dma_start(out=pb1[:], in_=pb1_d[:])
            pw2 = cp.tile([C // 2, OUT], dtype=f32)
            nc.sync.dma_start(out=pw2[:], in_=pw2_d[:])
            pb2 = cp.tile([OUT, 1], dtype=f32)
            nc.sync.dma_start(out=pb2[:], in_=pb2_d[:])

            z1p = pt.tile([C // 2, B], dtype=f32, space="PSUM", tag="tp")
            nc.tensor.matmul(z1p[:], lhsT=pw1[:], rhs=pooled[:], start=True, stop=True)
            z1 = wp.tile([C // 2, B], dtype=f32, tag="z1")
            nc.scalar.activation(z1[:], z1p[:], AFT.Relu, bias=pb1[:, 0:1])
            z2p = pt.tile([OUT, B], dtype=f32, space="PSUM", tag="tp")
            nc.tensor.matmul(z2p[:], lhsT=pw2[:], rhs=z1[:], start=True, stop=True)
            z2 = wp.tile([OUT, B], dtype=f32, tag="z2")
            nc.vector.tensor_scalar(out=z2[:], in0=z2p[:], scalar1=pb2[:, 0:1],
                                    scalar2=None, op0=ALU.add)
            nc.sync.dma_start(out=out_d[:], in_=z2[:])

    nc.compile()
    return nc


# ======================================================================
# entry point
# ======================================================================

def kernel(x, edge_index, batch, W0, b0, as0, ad0, W1, b1, as1, ad1,
           W2, b2, as2, ad2, pW1, pb1, pW2, pb2):
    x = np.asarray(x, dtype=F32)
    edge_index = np.asarray(edge_index)
    batch = np.asarray(batch)

    static, percore = preprocess(x, edge_index, batch)

    waug = {0: make_waug(np.asarray(W0, F32), np.asarray(as0, F32), np.asarray(ad0, F32)),
            1: make_waug(np.asarray(W1, F32), np.asarray(as1, F32), np.asarray(ad1, F32)),
            2: make_waug(np.asarray(W2, F32), np.asarray(as2, F32), np.asarray(ad2, F32))}
    b0r = np.broadcast_to(np.asarray(b0, F32), (PBLK, HC)).copy()
    b1r = np.broadcast_to(np.asarray(b1, F32), (PBLK, HC)).copy()
    b2r = np.broadcast_to(np.asarray(b2, F32), (PBLK, C)).copy()
    iota64 = np.broadcast_to(np.arange(B, dtype=F32), (PBLK, B)).copy()

    nc = build_program(static)

    import ml_dtypes
    bf = ml_dtypes.bfloat16
    waug = {k: v.astype(bf) for k, v in waug.items()}
    from concourse.bass_utils import run_bass_kernel_spmd
    in_maps = []
    for c in range(NCOR):
        in_maps.append(dict(
            xT0=percore["xT0"][c].astype(bf),
            Waug0=waug[0], Waug1=waug[1], Waug2=waug[2],
            b0rep=b0r, b1rep=b1r, b2rep=b2r,
            idx_all=percore["idx_all"][c],
            maskmul=percore["maskmul"][c],
            batchcol=percore["batchcol"][c],
            invcnt=percore["invcnt"][c],
            iota64=iota64,
            pW1=np.asarray(pW1, F32), pb1=np.asarray(pb1, F32).reshape(-1, 1),
            pW2=np.asarray(pW2, F32), pb2=np.asarray(pb2, F32).reshape(-1, 1),
        ))
    import os as _os
    trace = _os.environ.get("GAT_TRACE", "0") == "1"
    kw = {}
    if trace:
        kw = dict(trace=True, tmpdir=_os.environ.get("GAT_TRACE_DIR") or None)
    res = run_bass_kernel_spmd(nc, in_maps, list(range(NCOR)), **kw)
    if trace:
        print(f"HW exec time: {res.exec_time_ns} ns")
    out_t = res.results[0]["out_t"]            # [OUT, B]
    return np.ascontiguousarray(out_t.T).astype(F32)

